# revision 23
# baseline (speedup 1.0000x reference)
"""Trainium2 Bass kernel for nn_DeepBSpline (per-channel uniform-knot linear
B-spline activation with linear extrapolation).

Approach: the whole op (clamp + bin + two gathers + lerp + extrapolation) is,
per channel, a single continuous piecewise-linear function of x whose kinks
sit at the compile-time-known knot grid.  The host compresses the coefficient
table into its minimal relu basis

    f_c(x) = alpha_c + beta_c * x + sum_j D_cj * relu(x - b_cj)

keeping only kinks with a non-negligible slope change.

Fast paths for T == 1 (a 1-kink PWL):
- max: when the kink is at 0, alpha == 0 and the right slope is exactly 1
  (true for the leaky-relu-initialized table), f_c(x) = max(beta_c*x, x) —
  a single all-fp16 DVE scalar_tensor_tensor (mult, max) per tile, since a
  convex 1-kink PWL is the max of its two lines.
- relu1: generally, f_c(x) = [beta_c*x + Relu(D_c*x - D_c*b_c)] + alpha_c
  for D_c >= 0 — one ScalarE Relu (per-partition scale/bias APs) plus one
  DVE scalar_tensor_tensor, with an optional alpha add.

I/O is fp16: the op is memory-bound (256 MiB in + 256 MiB out at fp32 sits
exactly on the 8-core 360 GB/s-per-core DMA roofline), so the host casts x
to fp16 and the device streams fp16 both ways, halving HBM traffic; engines
compute in fp32 internally and the host upcasts the result (~2e-4 rel-l2).

Sharding: data-parallel over the batch dim — 8 cores x 2 batches each; each
core's (2, 64, 256, 256) slab is viewed as [128 partitions, 65536] with
partition p = b*64 + c, so per-channel constants become per-partition scalars.
"""

import os
import sys

import numpy as np

for _p in ("/opt/trn_rl_repo", "/root/.axon_site", "/root/.axon_site/_ro/trn_rl_repo",
           "/root/.axon_site/_ro/pypackages"):
    if os.path.isdir(_p) and _p not in sys.path:
        sys.path.append(_p)

GRID = 0.16
SIZE = 51
HALF = SIZE // 2
C = 64
N_BATCH = 16
HW = 256 * 256
N_CORES = 8
P = 128                      # partitions = 2 batches x 64 channels
BATCH_PER_CORE = N_BATCH // N_CORES
FREE = BATCH_PER_CORE * C * HW // P   # 65536 free-dim elements per partition
F_TILE = 4096
IO_DTYPE = os.environ.get("BSPLINE_IO_DTYPE", "float16")  # fp16 halves HBM traffic


def _build_pwl(coefficients_vect, tol_rel=1e-4):
    """Compress the spline table to relu-basis PWL coefficients (float64).

    Returns alpha[C], beta[C], terms (per channel list of (kink_x, slope_delta)),
    and the max term count across channels.
    """
    cv = np.asarray(coefficients_vect, np.float64).reshape(C, SIZE)
    slopes_x = np.diff(cv, axis=1) / GRID          # (C, 50) per-bin slopes
    dd = np.diff(slopes_x, axis=1)                 # (C, 49) slope changes at knots 1..49
    scale = np.abs(dd).max() + 1e-30
    keep = np.abs(dd) > tol_rel * scale
    alpha = np.empty(C)
    beta = np.empty(C)
    terms = []
    max_terms = 0
    for c in range(C):
        ks = [0] + list(np.nonzero(keep[c])[0] + 1) + [SIZE - 1]
        # refit chords so the PWL interpolates the exact table values at the
        # kept kinks and both endpoints
        k0, k1 = ks[0], ks[1]
        s0 = (cv[c, k1] - cv[c, k0]) / ((k1 - k0) * GRID)
        beta[c] = s0
        alpha[c] = cv[c, k0] - (k0 - HALF) * GRID * s0
        t = []
        prev_s = s0
        for i in range(1, len(ks) - 1):
            ka, kb = ks[i], ks[i + 1]
            s = (cv[c, kb] - cv[c, ka]) / ((kb - ka) * GRID)
            t.append(((ka - HALF) * GRID, s - prev_s))
            prev_s = s
        terms.append(t)
        max_terms = max(max_terms, len(t))
    return alpha, beta, terms, max_terms


def _consts_array(alpha, beta, terms, T):
    """[P, 2+2T] float32: per partition (b*64+c): alpha, beta, (-b_j, D_j)*T."""
    K = 2 + 2 * T
    a = np.zeros((C, K), np.float32)
    a[:, 0] = np.asarray(alpha, np.float32)
    a[:, 1] = np.asarray(beta, np.float32)
    for c in range(C):
        for j, (b, d) in enumerate(terms[c]):
            a[c, 2 + 2 * j] = np.float32(-b)
            a[c, 3 + 2 * j] = np.float32(d)
    return np.tile(a, (P // C, 1)).astype(np.float32)


def _relu1_params(alpha, beta, terms):
    """Single-relu decomposition for T==1 with D >= 0.

    f(x) = alpha + beta*x + D*relu(x - b)
         = [ beta*x + Relu(D*x - D*b) ] + alpha          (D >= 0)

    Returns (consts[P,4], with_alpha) or None; columns: D, -D*b, alpha, beta.
    """
    b = np.array([t[0][0] if t else 0.0 for t in terms])
    D = np.array([t[0][1] if t else 0.0 for t in terms])
    alpha = np.asarray(alpha)
    beta = np.asarray(beta)
    if not np.all(D >= 0.0):        # D == 0 (no kink) degenerates to rt = 0
        return None
    arr = np.stack([D, -D * b, alpha, beta], axis=1).astype(np.float32)  # (C,4)
    consts = np.tile(arr, (P // C, 1)).astype(np.float32)
    with_alpha = bool(np.any(np.abs(alpha) > 1e-7 * (np.abs(beta).max() + 1.0)))
    return consts, with_alpha


def _max_params(alpha, beta, terms):
    """Two-line max decomposition: needs T==1, b==0, alpha==0, beta+D==1.

    Then f(x) = max(beta*x, x) exactly (a 1-kink convex PWL is the max of
    its two lines; here line2 is y=x).  Returns (consts[P,1] or None,
    beta_imm or None): when every channel shares the same beta, beta_imm is
    that scalar and consts is None (the program bakes it as an immediate and
    needs no consts tensor at all); otherwise consts carries per-partition
    beta.  Returns (None, None) if the decomposition doesn't apply.
    """
    b = np.array([t[0][0] if t else 0.0 for t in terms])
    D = np.array([t[0][1] if t else 0.0 for t in terms])
    alpha = np.asarray(alpha)
    beta = np.asarray(beta)
    s = beta + D
    scale = np.abs(beta).max() + 1.0
    ok = (np.all(D > 0) and np.abs(b).max() < 1e-9
          and np.abs(alpha).max() < 1e-9 * scale
          and np.abs(s - 1.0).max() < 1e-9)
    if not ok:
        return None, None
    beta32 = beta.astype(np.float32)
    if beta32.max() == beta32.min():
        return None, float(beta32[0])
    consts = np.tile(beta32[:, None], (P // C, 1))
    return np.ascontiguousarray(consts, dtype=np.float32), None


def _max_tile_sizes(free=FREE, f_tile=F_TILE, split_edges=True):
    """Tile size schedule.  The first/last full tile are split into small
    chunks so the first store launches ~4 us earlier (shorter pipeline fill)
    and the final store's tail is ~0.7 us instead of ~2.9 us (shorter drain);
    steady-state HBM traffic is unchanged."""
    n_tiles = free // f_tile
    assert n_tiles * f_tile == free
    if not split_edges or n_tiles < 3:
        return [f_tile] * n_tiles
    head = [f_tile // 4] * 4
    tail = [f_tile // 2, f_tile // 4, f_tile // 4]
    return head + [f_tile] * (n_tiles - 2) + tail


def _build_bass_max(free=FREE, f_tile=F_TILE, repeat=1, io_dtype=IO_DTYPE,
                    split_edges=True, beta_imm=None):
    """Single-DVE-op path: out = max(beta*x, x) per tile, all io_dtype.

    beta_imm: when set, beta is baked as an immediate — no consts tensor, no
    consts DMA, nothing gating the first compute but the first x chunk.
    """
    from contextlib import ExitStack

    import concourse.tile as tile
    from concourse import bacc, mybir

    nc = bacc.Bacc("TRN2", target_bir_lowering=False, debug=False,
                   num_devices=N_CORES)
    f32 = mybir.dt.float32
    fio = getattr(mybir.dt, io_dtype)
    x_d = nc.dram_tensor("x", [P, free], fio, kind="ExternalInput")
    c_d = (None if beta_imm is not None else
           nc.dram_tensor("consts", [P, 1], f32, kind="ExternalInput"))
    o_d = nc.dram_tensor("out", [P, free], fio, kind="ExternalOutput")
    sizes = _max_tile_sizes(free, f_tile, split_edges)
    assert sum(sizes) == free

    mul = mybir.AluOpType.mult
    mx = mybir.AluOpType.max

    with tile.TileContext(nc) as tc, ExitStack() as ctx:
        if c_d is not None:
            cpool = ctx.enter_context(tc.tile_pool(name="cpool", bufs=1))
            ct = cpool.tile([P, 1], f32)
            nc.sync.dma_start(ct[:], c_d.ap())
            beta_op = ct[:, 0:1]
        else:
            beta_op = float(beta_imm)

        xin = ctx.enter_context(tc.tile_pool(name="xin", bufs=6))
        op = ctx.enter_context(tc.tile_pool(name="op", bufs=6))

        for _r in range(repeat):
            off = 0
            for sz in sizes:
                xt = xin.tile([P, sz], fio)
                # loads on qACT (ACT HWDGE), stores on qSP: one direction per
                # hardware queue so neither head-of-line-blocks the other
                nc.scalar.dma_start(xt[:], x_d.ap()[:, off:off + sz])

                ot = op.tile([P, sz], fio)
                nc.vector.scalar_tensor_tensor(ot[:], xt[:], beta_op, xt[:],
                                               mul, mx)

                nc.sync.dma_start(o_d.ap()[:, off:off + sz], ot[:])
                off += sz

    nc.compile()
    return nc


def _build_bass_relu1(with_alpha, free=FREE, f_tile=F_TILE, repeat=1,
                      io_dtype=IO_DTYPE):
    """T==1 fast path with proven ops only.

    Per tile: ScalarE rt = Relu(D*x - D*b) (per-partition scale/bias APs),
    then one DVE scalar_tensor_tensor out = beta*x + rt, all io_dtype
    operands so 16-bit hits the DVE 2x perf mode.  Optional + alpha.
    """
    from contextlib import ExitStack

    import concourse.bass as bass
    import concourse.tile as tile
    from concourse import bacc, mybir

    nc = bacc.Bacc("TRN2", target_bir_lowering=False, debug=False,
                   num_devices=N_CORES)
    f32 = mybir.dt.float32
    fio = getattr(mybir.dt, io_dtype)
    x_d = nc.dram_tensor("x", [P, free], fio, kind="ExternalInput")
    c_d = nc.dram_tensor("consts", [P, 4], f32, kind="ExternalInput")
    o_d = nc.dram_tensor("out", [P, free], fio, kind="ExternalOutput")
    n_tiles = free // f_tile
    assert n_tiles * f_tile == free

    mul = mybir.AluOpType.mult
    add = mybir.AluOpType.add
    relu = mybir.ActivationFunctionType.Relu

    with tile.TileContext(nc) as tc, ExitStack() as ctx:
        cpool = ctx.enter_context(tc.tile_pool(name="cpool", bufs=1))
        ct = cpool.tile([P, 4], f32)
        nc.sync.dma_start(ct[:], c_d.ap())

        xin = ctx.enter_context(tc.tile_pool(name="xin", bufs=4))
        rp = ctx.enter_context(tc.tile_pool(name="rp", bufs=3))
        op = ctx.enter_context(tc.tile_pool(name="op", bufs=4))
        op2 = ctx.enter_context(tc.tile_pool(name="op2", bufs=4)) if with_alpha else None

        for _r in range(repeat):
            for i in range(n_tiles):
                xt = xin.tile([P, f_tile], fio)
                nc.scalar.dma_start(xt[:], x_d.ap()[:, bass.ts(i, f_tile)])

                rt = rp.tile([P, f_tile], fio)
                nc.scalar.activation(rt[:], xt[:], relu,
                                     bias=ct[:, 1:2], scale=ct[:, 0:1])
                ot = op.tile([P, f_tile], fio)
                nc.vector.scalar_tensor_tensor(ot[:], xt[:], ct[:, 3:4], rt[:],
                                               mul, add)
                if with_alpha:
                    o2 = op2.tile([P, f_tile], fio)
                    nc.vector.tensor_scalar(o2[:], ot[:], ct[:, 2:3], None, add)
                    ot = o2

                nc.sync.dma_start(o_d.ap()[:, bass.ts(i, f_tile)], ot[:])

    nc.compile()
    return nc


def _build_bass(T, free=FREE, f_tile=F_TILE, repeat=1, io_dtype=IO_DTYPE):
    """Generic relu-basis program for term count T (fallback path).

    All DVE operands are io_dtype so 16-bit runs hit the 2x DVE perf mode.
    """
    from contextlib import ExitStack

    import concourse.bass as bass
    import concourse.tile as tile
    from concourse import bacc, mybir

    nc = bacc.Bacc("TRN2", target_bir_lowering=False, debug=False,
                   num_devices=N_CORES)
    f32 = mybir.dt.float32
    fio = getattr(mybir.dt, io_dtype)
    x_d = nc.dram_tensor("x", [P, free], fio, kind="ExternalInput")
    c_d = nc.dram_tensor("consts", [P, 2 + 2 * T], f32, kind="ExternalInput")
    o_d = nc.dram_tensor("out", [P, free], fio, kind="ExternalOutput")
    n_tiles = free // f_tile
    assert n_tiles * f_tile == free

    mul = mybir.AluOpType.mult
    add = mybir.AluOpType.add
    relu = mybir.ActivationFunctionType.Relu

    with tile.TileContext(nc) as tc, ExitStack() as ctx:
        cpool = ctx.enter_context(tc.tile_pool(name="cpool", bufs=1))
        ct = cpool.tile([P, 2 + 2 * T], f32)
        nc.sync.dma_start(ct[:], c_d.ap())

        xin = ctx.enter_context(tc.tile_pool(name="xin", bufs=4))
        fp = ctx.enter_context(tc.tile_pool(name="fp", bufs=2))
        rp = ctx.enter_context(tc.tile_pool(name="rp", bufs=2))
        op = ctx.enter_context(tc.tile_pool(name="op", bufs=3))

        for _r in range(repeat):
            for i in range(n_tiles):
                xt = xin.tile([P, f_tile], fio)
                nc.scalar.dma_start(xt[:], x_d.ap()[:, bass.ts(i, f_tile)])

                acc = fp.tile([P, f_tile], fio)
                nc.vector.tensor_scalar(acc[:], xt[:], ct[:, 1:2], ct[:, 0:1],
                                        mul, add)

                for j in range(T):
                    rt = rp.tile([P, f_tile], fio)
                    nc.scalar.activation(rt[:], xt[:], relu,
                                         bias=ct[:, 2 + 2 * j:3 + 2 * j])
                    ot = op.tile([P, f_tile], fio)
                    nc.vector.scalar_tensor_tensor(ot[:], rt[:],
                                                   ct[:, 3 + 2 * j:4 + 2 * j],
                                                   acc[:], mul, add)
                    acc = ot

                nc.sync.dma_start(o_d.ap()[:, bass.ts(i, f_tile)], acc[:])

    nc.compile()
    return nc


_NC_CACHE = {}


def _get_nc_relu1(with_alpha, repeat=1):
    key = ("relu1", with_alpha, repeat)
    if key not in _NC_CACHE:
        _NC_CACHE[key] = _build_bass_relu1(with_alpha, repeat=repeat)
    return _NC_CACHE[key]


def _get_nc_max(repeat=1, beta_imm=None):
    key = ("max", repeat, None if beta_imm is None else round(beta_imm, 12))
    if key not in _NC_CACHE:
        _NC_CACHE[key] = _build_bass_max(repeat=repeat, beta_imm=beta_imm)
    return _NC_CACHE[key]


def _get_nc(T, repeat=1):
    key = ("gen", T, repeat)
    if key not in _NC_CACHE:
        _NC_CACHE[key] = _build_bass(T, repeat=repeat)
    return _NC_CACHE[key]


def _plan(coefficients_vect):
    """Decide program + consts for these coefficients.

    Returns (kind, nc_getter(repeat), consts), kind in {'max','relu1','gen'}.
    """
    alpha, beta, terms, T = _build_pwl(coefficients_vect)
    T = max(T, 1)
    if T == 1:
        mx_consts, beta_imm = _max_params(alpha, beta, terms)
        if beta_imm is not None:
            return ("max_imm",
                    lambda repeat=1: _get_nc_max(repeat, beta_imm=beta_imm),
                    None)
        if mx_consts is not None:
            return ("max", lambda repeat=1: _get_nc_max(repeat), mx_consts)
        fast = _relu1_params(alpha, beta, terms)
        if fast is not None:
            consts, with_alpha = fast
            return ("relu1",
                    lambda repeat=1: _get_nc_relu1(with_alpha, repeat),
                    consts)
    consts = _consts_array(alpha, beta, terms, T)
    return ("gen", lambda repeat=1: _get_nc(T, repeat), consts)


def _make_in_maps(x, consts):
    np_io = np.float16 if IO_DTYPE == "float16" else np.float32
    xc = np.ascontiguousarray(np.asarray(x).astype(np_io))
    maps = []
    for i in range(N_CORES):
        m = {"x": xc[i * BATCH_PER_CORE:(i + 1) * BATCH_PER_CORE].reshape(P, FREE)}
        if consts is not None:
            m["consts"] = consts
        maps.append(m)
    return maps


def kernel(x, coefficients_vect, size):
    assert int(size) == SIZE
    x = np.asarray(x)
    assert x.shape == (N_BATCH, C, 256, 256)
    cv = np.asarray(coefficients_vect, np.float32)

    kind, get_nc, consts = _plan(cv)

    from concourse.bass_utils import run_bass_kernel_spmd

    nc = get_nc()
    in_maps = _make_in_maps(x, consts)
    res = run_bass_kernel_spmd(nc, in_maps, list(range(N_CORES))).results
    out = np.concatenate(
        [r["out"].reshape(BATCH_PER_CORE, C, 256, 256) for r in res], axis=0
    )
    return out.astype(np.float32)
